# revision 8
# baseline (speedup 1.0000x reference)
"""Trainium2 Bass kernel for nn_Decoder_30777735643309.

GRU decoder: ses = tanh(lin1(ses_encoding)); 50 sequential GRU steps with
hidden input concat(h, ses); per-step logits over a 10004 vocab.

Strategy (8 cores, no collectives): data-parallel over batch (16 rows/core),
transposed on-chip layout (features on partitions, batch/time in the free
dim).  Algebraic splits:
  1. Hfull = [h, ses] with ses constant -> gh = h @ Whh[:, :H].T + CT where
     CT = ses @ Whh[:, H:].T + bhh is computed once.
  2. Critical-path split: only the first H columns of each gate (r,z,n) feed
     the recurrent state h' = hnew[:, :H].  The gate rows are permuted on the
     host so those 12 chunks are contiguous; only they are computed inside
     the sequential loop (halving the per-step weight traffic).  The other
     12 chunks (needed only for hnew[:, H:] -> logits) are recomputed
     afterwards as one batched matmul over all 50 steps.
  3. For the deferred r/z gates, gx = emb @ Wih.T is accumulated directly
     into the same PSUM group as gh (no gx store for those chunks).
Matmul inputs are bf16 (fp32 PSUM accumulation); logits are written bf16 and
upcast on the host.
"""

import numpy as np
import ml_dtypes

import concourse.bacc as bacc
import concourse.mybir as mybir
import concourse.tile as tile
from concourse.bass import IndirectOffsetOnAxis
from concourse.bass_utils import run_bass_kernel_spmd
from concourse.masks import make_identity

F32 = mybir.dt.float32
BF16 = mybir.dt.bfloat16
I32 = mybir.dt.int32
AF = mybir.ActivationFunctionType
OP = mybir.AluOpType

V = 10004
E = 300
EP = 384          # E padded to 3 K-chunks of 128
SH = 1024
H = 512
G = 1024          # GRU hidden = 2*H
G3 = 3 * G        # 3072
B, T = 128, 50
NCORES = 8
BL = B // NCORES  # 16 batch rows per core
NT = T * BL       # 800 (t-major columns: col = t*BL + b)
NTP = 896         # NT padded to 7 chunks of 128 (DRAM out rows)
KH = H // 128     # 4 K-chunks for the h-part matmul
M3 = G3 // 128    # 24 feature chunks of the gate dim
MC = 12           # critical chunks (r_c, z_c, n_c)
NB = 2            # column blocking for the batched matmuls
NBW = NT // NB    # 400 columns per block
NV = 20           # vocab blocks of 512

# permutation of the 3G gate dim: crit-first
# new order: r[:H], z[:H], n[:H], r[H:], z[H:], n[H:]
PERM = np.r_[0:H, G:G + H, 2 * G:2 * G + H,
             H:G, G + H:2 * G, 2 * G + H:3 * G]


def build_program(reps: int = 1, debug: bool = False, loop: bool = False):
    nc = bacc.Bacc()

    # ---- DRAM I/O ----
    d_sesenc = nc.dram_tensor("sesenc", [SH, BL], BF16, kind="ExternalInput")
    d_xw = nc.dram_tensor("xw", [128, 7], I32, kind="ExternalInput")
    d_emb = nc.dram_tensor("emb", [V, E], F32, kind="ExternalInput")
    d_whh_h = nc.dram_tensor("whh_h", [H, G3], BF16, kind="ExternalInput")
    d_whh_s = nc.dram_tensor("whh_s", [H, G3], BF16, kind="ExternalInput")
    d_wih = nc.dram_tensor("wih", [EP, G3], BF16, kind="ExternalInput")
    d_w1 = nc.dram_tensor("w1", [SH, H], BF16, kind="ExternalInput")
    d_w2 = nc.dram_tensor("w2", [G, E], BF16, kind="ExternalInput")
    d_wout = nc.dram_tensor("wout", [EP, V], BF16, kind="ExternalInput")
    d_b1t = nc.dram_tensor("b1t", [128, H // 128], F32, kind="ExternalInput")
    d_biht = nc.dram_tensor("biht", [128, M3], F32, kind="ExternalInput")
    d_bhht = nc.dram_tensor("bhht", [128, M3], F32, kind="ExternalInput")
    d_b2t = nc.dram_tensor("b2t", [128, EP // 128], F32, kind="ExternalInput")
    # t-major rows (row = t*BL + b); rows NT..NTP are junk; host reorders
    d_out = nc.dram_tensor("out", [NTP, V], BF16, kind="ExternalOutput")

    with tile.TileContext(nc) as tc:
        import contextlib
        with contextlib.ExitStack() as ctx:
            persist = ctx.enter_context(tc.tile_pool(name="persist", bufs=1))
            step = ctx.enter_context(tc.tile_pool(name="step", bufs=2))
            post = ctx.enter_context(tc.tile_pool(name="post", bufs=2))
            psG = ctx.enter_context(tc.tile_pool(name="psG", bufs=3, space="PSUM"))
            psMM = ctx.enter_context(tc.tile_pool(name="psMM", bufs=4, space="PSUM"))
            psT = ctx.enter_context(tc.tile_pool(name="psT", bufs=1, space="PSUM"))
            woutp = ctx.enter_context(tc.tile_pool(name="woutp", bufs=3))
            lout = ctx.enter_context(tc.tile_pool(name="lout", bufs=2))

            # persistent SBUF tensors (weights loaded once, outside rep loop)
            whh_h_sb = persist.tile([128, KH, G3], BF16)
            whh_s_sb = persist.tile([128, KH, G3], BF16)
            wih_sb = persist.tile([128, EP // 128, G3], BF16)
            w1_sb = persist.tile([128, SH // 128, H], BF16)
            w2_sb = persist.tile([128, G // 128, E], BF16)
            b1t = persist.tile([128, H // 128], F32)
            biht = persist.tile([128, M3], F32)
            bhht = persist.tile([128, M3], F32)
            b2t = persist.tile([128, EP // 128], F32)
            ident = persist.tile([128, 128], F32)
            # per-rep working tensors
            gxc = persist.tile([128, MC, NT], BF16)      # crit gx (+ct+bih for rz, +bih for n)
            gxn2 = persist.tile([128, 4, NT], BF16)      # noncrit n gx (+bih)
            embxT = persist.tile([128, EP // 128, NT], BF16)
            hsT = persist.tile([128, KH, NT + 2 * BL], BF16)  # states s_0..s_50
            hnT_nc = persist.tile([128, KH, NT], BF16)   # hnew[:, H:] per step
            oT = persist.tile([128, EP // 128, NT], BF16)
            ct = persist.tile([128, M3, BL], F32)
            sesT_bf = persist.tile([128, KH, BL], BF16)

            nc.sync.dma_start(out=whh_h_sb, in_=d_whh_h[:, :].rearrange("(k p) c -> p k c", p=128))
            nc.sync.dma_start(out=whh_s_sb, in_=d_whh_s[:, :].rearrange("(k p) c -> p k c", p=128))
            nc.sync.dma_start(out=wih_sb, in_=d_wih[:, :].rearrange("(k p) c -> p k c", p=128))
            nc.sync.dma_start(out=w1_sb, in_=d_w1[:, :].rearrange("(k p) c -> p k c", p=128))
            nc.sync.dma_start(out=w2_sb, in_=d_w2[:, :].rearrange("(k p) c -> p k c", p=128))
            nc.sync.dma_start(out=b1t, in_=d_b1t[:, :])
            nc.sync.dma_start(out=biht, in_=d_biht[:, :])
            nc.sync.dma_start(out=bhht, in_=d_bhht[:, :])
            nc.sync.dma_start(out=b2t, in_=d_b2t[:, :])
            make_identity(nc, ident)

            import contextlib as _ctxlib

            if loop:
                loop_cm = tc.For_i(0, reps, 1)
                rep_iter = [0]
            else:
                loop_cm = _ctxlib.nullcontext()
                rep_iter = range(reps)

            with loop_cm:
              for _rep in rep_iter:
                with tc.tile_pool(name=f"setup_{_rep}", bufs=1) as setup, \
                     tc.tile_pool(name=f"gatherp_{_rep}", bufs=3) as gatherp:
                    sesenc_sb = setup.tile([128, SH // 128, BL], BF16)
                    xw = setup.tile([128, 7], I32)
                    nc.sync.dma_start(out=sesenc_sb, in_=d_sesenc[:, :].rearrange("(k p) c -> p k c", p=128))
                    nc.sync.dma_start(out=xw, in_=d_xw[:, :])

                    with nc.named_scope("ses"):
                        # ses = tanh(W1 @ ses_encT + b1)  -> [H, BL] as [128, 4, BL]
                        ps_s = psT.tile([128, KH, BL], F32, tag="tp")
                        for m in range(KH):
                            for k in range(SH // 128):
                                nc.tensor.matmul(
                                    out=ps_s[:, m, :],
                                    lhsT=w1_sb[:, k, m * 128:(m + 1) * 128],
                                    rhs=sesenc_sb[:, k, :],
                                    start=(k == 0), stop=(k == SH // 128 - 1))
                        for m in range(KH):
                            nc.scalar.activation(sesT_bf[:, m, :], ps_s[:, m, :], AF.Tanh,
                                                 bias=b1t[:, m:m + 1])
                        # initial state s_0 = ses
                        nc.vector.tensor_copy(hsT[:, :, 0:BL], sesT_bf)

                        # CT = Whh_ses @ sesT + bhh  -> [3G, BL]
                        ps_gs = psT.tile([128, M3, BL], F32, tag="tp")
                        for m in range(M3):
                            for k in range(KH):
                                nc.tensor.matmul(
                                    out=ps_gs[:, m, :],
                                    lhsT=whh_s_sb[:, k, m * 128:(m + 1) * 128],
                                    rhs=sesT_bf[:, k, :],
                                    start=(k == 0), stop=(k == KH - 1))
                        nc.vector.tensor_tensor(
                            out=ct, in0=ps_gs,
                            in1=bhht[:, :, None].broadcast_to([128, M3, BL]), op=OP.add)

                    # zero the padded K rows (chunk 2 covers E rows 256..384;
                    # rows 300.. must be 0 so matmuls against them stay finite)
                    nc.vector.memset(embxT[:, 2, :], 0.0)
                    nc.vector.memset(oT[:, 2, :], 0.0)

                    with nc.named_scope("gather"):
                        # gather emb[x] -> transpose -> embxT [EP, NT] bf16
                        for c in range(7):
                            pm = 128 if c < 6 else NT - 6 * 128
                            embx_c = gatherp.tile([128, E], F32, tag="gx")
                            nc.gpsimd.indirect_dma_start(
                                out=embx_c[:pm, :], out_offset=None,
                                in_=d_emb[:, :],
                                in_offset=IndirectOffsetOnAxis(ap=xw[:pm, c:c + 1], axis=0))
                            for k in range(EP // 128):
                                kw = min(128, E - k * 128)
                                if kw <= 0:
                                    break
                                cw = min(128, NT - c * 128)
                                ps_t = psMM.tile([128, 128], F32, tag="mm")
                                nc.tensor.transpose(
                                    out=ps_t[:kw, :pm],
                                    in_=embx_c[:pm, k * 128:k * 128 + kw],
                                    identity=ident[:pm, :pm])
                                nc.vector.tensor_copy(
                                    embxT[:kw, k, c * 128:c * 128 + cw], ps_t[:kw, :cw])

                    with nc.named_scope("gx"):
                        # gx for crit chunks (0..11) and noncrit-n chunks (20..23)
                        # rz crit (0..7):  gx + bih + ct
                        # n chunks:        gx + bih
                        for mi, m in enumerate(list(range(MC)) + [20, 21, 22, 23]):
                            for nb in range(NB):
                                cs = slice(nb * NBW, (nb + 1) * NBW)
                                ps_gx = psMM.tile([128, NBW], F32, tag="mm")
                                for k in range(EP // 128):
                                    nc.tensor.matmul(
                                        out=ps_gx,
                                        lhsT=wih_sb[:, k, m * 128:(m + 1) * 128],
                                        rhs=embxT[:, k, cs],
                                        start=(k == 0), stop=(k == EP // 128 - 1))
                                dst = (gxc[:, mi, cs] if mi < MC
                                       else gxn2[:, mi - MC, cs])
                                if m < 8:  # rz crit: + bih + ct
                                    nc.vector.scalar_tensor_tensor(
                                        out=dst.rearrange("p (t b) -> p t b", b=BL),
                                        in0=ps_gx[:, :].rearrange("p (t b) -> p t b", b=BL),
                                        scalar=biht[:, m:m + 1],
                                        in1=ct[:, m, None, :].broadcast_to([128, NBW // BL, BL]),
                                        op0=OP.add, op1=OP.add)
                                else:      # n chunks: + bih
                                    nc.vector.tensor_scalar_add(
                                        out=dst, in0=ps_gx, scalar1=biht[:, m:m + 1])

                # ---- recurrence ----
                # chunk roles (permuted): 0-3 r_c, 4-7 z_c, 8-11 n_c
                with nc.named_scope("recur"):
                    for t in range(T):
                        ts = slice(t * BL, (t + 1) * BL)
                        ts1 = slice((t + 1) * BL, (t + 2) * BL)
                        ps_z = psG.tile([128, 4, BL], F32, tag="g")
                        ps_r = psG.tile([128, 4, BL], F32, tag="g")
                        ps_n = psG.tile([128, 4, BL], F32, tag="g")
                        # order: z (4..7), r (0..3), n (8..11) in separate
                        # PSUM banks so gate math overlaps the matmul stream
                        for m in (4, 5, 6, 7, 0, 1, 2, 3, 8, 9, 10, 11):
                            ps = (ps_z[:, m - 4, :] if m >= 4 and m < 8 else
                                  ps_r[:, m, :] if m < 4 else ps_n[:, m - 8, :])
                            for k in range(KH):
                                nc.tensor.matmul(
                                    out=ps,
                                    lhsT=whh_h_sb[:, k, m * 128:(m + 1) * 128],
                                    rhs=hsT[:, k, ts],
                                    start=(k == 0), stop=(k == KH - 1))
                        zp = step.tile([128, 4, BL], F32, tag="zp")
                        nc.vector.tensor_tensor(out=zp, in0=ps_z,
                                                in1=gxc[:, 4:8, ts], op=OP.add)
                        zz = step.tile([128, 4, BL], F32, tag="zz")
                        nc.scalar.activation(zz, zp, AF.Sigmoid)
                        # a = z*h ; bz = 1-z  (off critical path)
                        a = step.tile([128, 4, BL], F32, tag="a")
                        nc.vector.tensor_tensor(out=a, in0=zz, in1=hsT[:, :, ts],
                                                op=OP.mult)
                        bz = step.tile([128, 4, BL], F32, tag="bz")
                        nc.vector.tensor_scalar(out=bz, in0=zz, scalar1=-1.0,
                                                scalar2=1.0, op0=OP.mult, op1=OP.add)
                        rp = step.tile([128, 4, BL], F32, tag="rp")
                        nc.vector.tensor_tensor(out=rp, in0=ps_r,
                                                in1=gxc[:, 0:4, ts], op=OP.add)
                        rr = step.tile([128, 4, BL], F32, tag="rr")
                        nc.scalar.activation(rr, rp, AF.Sigmoid)
                        ghn = step.tile([128, 4, BL], F32, tag="ghn")
                        nc.vector.tensor_tensor(out=ghn, in0=ps_n, in1=ct[:, 8:MC, :],
                                                op=OP.add)
                        t1 = step.tile([128, 4, BL], F32, tag="t1")
                        nc.vector.tensor_tensor(out=t1, in0=rr, in1=ghn, op=OP.mult)
                        nc.vector.tensor_tensor(out=t1, in0=t1, in1=gxc[:, 8:MC, ts],
                                                op=OP.add)
                        nt = step.tile([128, 4, BL], F32, tag="nt")
                        nc.scalar.activation(nt, t1, AF.Tanh)
                        c_ = step.tile([128, 4, BL], F32, tag="c")
                        nc.vector.tensor_tensor(out=c_, in0=bz, in1=nt, op=OP.mult)
                        nc.vector.tensor_tensor(out=hsT[:, :, ts1], in0=a, in1=c_,
                                                op=OP.add)

                # ---- noncrit gates, batched over all t ----
                # chunks 12-15 r_n, 16-19 z_n, 20-23 n_n
                with nc.named_scope("noncrit"):
                    for nb in range(NB):
                        cs = slice(nb * NBW, (nb + 1) * NBW)
                        rznc = post.tile([128, 8, NBW], F32, tag="rznc", bufs=1)
                        for mi, m in enumerate(range(MC, M3)):
                            ps_nc = psMM.tile([128, NBW], F32, tag="mm")
                            # r_n/z_n: fold gx = Wih@emb into the same PSUM
                            # group.  n_n: gh only (gx enters outside r*(.),
                            # via gxn2).
                            if mi < 8:
                                for k in range(EP // 128):
                                    nc.tensor.matmul(
                                        out=ps_nc,
                                        lhsT=wih_sb[:, k, m * 128:(m + 1) * 128],
                                        rhs=embxT[:, k, cs],
                                        start=(k == 0), stop=False)
                            for k in range(KH):
                                nc.tensor.matmul(
                                    out=ps_nc,
                                    lhsT=whh_h_sb[:, k, m * 128:(m + 1) * 128],
                                    rhs=hsT[:, k, cs],
                                    start=(mi >= 8 and k == 0), stop=(k == KH - 1))
                            if mi < 8:  # r_n, z_n: sigma(ps + bih + ct)
                                tmp = post.tile([128, NBW], F32, tag="nctmp")
                                nc.vector.scalar_tensor_tensor(
                                    out=tmp.rearrange("p (t b) -> p t b", b=BL),
                                    in0=ps_nc[:, :].rearrange("p (t b) -> p t b", b=BL),
                                    scalar=biht[:, m:m + 1],
                                    in1=ct[:, m, None, :].broadcast_to([128, NBW // BL, BL]),
                                    op0=OP.add, op1=OP.add)
                                nc.scalar.activation(rznc[:, mi, :], tmp, AF.Sigmoid)
                            else:       # n_n: tanh(gxn2 + r*(ps + ct))
                                j = mi - 8
                                ghn_nc = post.tile([128, NBW], F32, tag="ghnnc")
                                nc.vector.tensor_tensor(
                                    out=ghn_nc.rearrange("p (t b) -> p t b", b=BL),
                                    in0=ps_nc[:, :].rearrange("p (t b) -> p t b", b=BL),
                                    in1=ct[:, m, None, :].broadcast_to([128, NBW // BL, BL]),
                                    op=OP.add)
                                t1n = post.tile([128, NBW], F32, tag="t1n")
                                nc.vector.tensor_tensor(out=t1n, in0=rznc[:, j, :],
                                                        in1=ghn_nc, op=OP.mult)
                                nc.vector.tensor_tensor(out=t1n, in0=t1n,
                                                        in1=gxn2[:, j, cs], op=OP.add)
                                ntn = post.tile([128, NBW], F32, tag="ntn")
                                nc.scalar.activation(ntn, t1n, AF.Tanh)
                                # hn_nc = (1-z)*n + z*ses
                                bzn = post.tile([128, NBW], F32, tag="bzn")
                                nc.vector.tensor_scalar(out=bzn, in0=rznc[:, 4 + j, :],
                                                        scalar1=-1.0, scalar2=1.0,
                                                        op0=OP.mult, op1=OP.add)
                                an = post.tile([128, NBW], F32, tag="an")
                                nc.vector.tensor_tensor(
                                    out=an.rearrange("p (t b) -> p t b", b=BL),
                                    in0=rznc[:, 4 + j, :].rearrange("p (t b) -> p t b", b=BL),
                                    in1=sesT_bf[:, j, None, :].broadcast_to([128, NBW // BL, BL]),
                                    op=OP.mult)
                                cn = post.tile([128, NBW], F32, tag="cn")
                                nc.vector.tensor_tensor(out=cn, in0=bzn, in1=ntn,
                                                        op=OP.mult)
                                nc.vector.tensor_tensor(out=hnT_nc[:, j, cs], in0=an,
                                                        in1=cn, op=OP.add)

                # ---- o = W2 @ hnew + b2 + embx ----
                with nc.named_scope("oproj"):
                    for m in range(EP // 128):
                        pm = min(128, E - m * 128)
                        for nb in range(NB):
                            cs = slice(nb * NBW, (nb + 1) * NBW)
                            css = slice(BL + nb * NBW, BL + (nb + 1) * NBW)
                            ps_o = psMM.tile([128, NBW], F32, tag="mm")
                            for k in range(G // 128):
                                if k < KH:   # crit half: shifted states s_{t+1}
                                    rhs = hsT[:, k, css]
                                else:        # noncrit half
                                    rhs = hnT_nc[:, k - KH, cs]
                                nc.tensor.matmul(
                                    out=ps_o[:pm, :],
                                    lhsT=w2_sb[:, k, m * 128:m * 128 + pm],
                                    rhs=rhs,
                                    start=(k == 0), stop=(k == G // 128 - 1))
                            nc.vector.scalar_tensor_tensor(
                                out=oT[:pm, m, cs], in0=ps_o[:pm, :],
                                scalar=b2t[:pm, m:m + 1],
                                in1=embxT[:pm, m, cs], op0=OP.add, op1=OP.add)

                if debug and _rep == 0:
                    dbg = {
                        "dbg_ses": ([128, KH * BL], BF16, sesT_bf),
                        "dbg_ct": ([128, M3 * BL], F32, ct),
                        "dbg_embx": ([128, (EP // 128) * NT], BF16, embxT),
                        "dbg_gxc": ([128, MC * NT], BF16, gxc),
                        "dbg_gxn2": ([128, 4 * NT], BF16, gxn2),
                        "dbg_hs": ([128, KH * (NT + 2 * BL)], BF16, hsT),
                        "dbg_hnnc": ([128, KH * NT], BF16, hnT_nc),
                        "dbg_o": ([128, (EP // 128) * NT], BF16, oT),
                    }
                    for nm, (shp, dt, tl) in dbg.items():
                        dh = nc.dram_tensor(nm, shp, dt, kind="ExternalOutput")
                        nc.sync.dma_start(out=dh[:, :], in_=tl[:, :].rearrange("p a b -> p (a b)"))

                # ---- logits = oT.T @ Wout.T -> DRAM ----
                with nc.named_scope("logits"):
                    for nv in range(NV):
                        nw = min(512, V - nv * 512)
                        wchunk = woutp.tile([128, EP // 128, 512], BF16, tag="w")
                        nc.sync.dma_start(
                            out=wchunk[:, :, :nw],
                            in_=d_wout[:, nv * 512:nv * 512 + nw].rearrange(
                                "(k p) v -> p k v", p=128))
                        lsb = lout.tile([128, 7, 512], BF16, tag="l")
                        for mt in range(7):
                            pm = 128 if mt < 6 else NT - 6 * 128
                            ms = slice(mt * 128, mt * 128 + pm)
                            ps_l = psMM.tile([128, 512], F32, tag="mm")
                            for k in range(EP // 128):
                                nc.tensor.matmul(
                                    out=ps_l[:pm, :nw],
                                    lhsT=oT[:, k, ms],
                                    rhs=wchunk[:, k, :nw],
                                    start=(k == 0), stop=(k == EP // 128 - 1))
                            if mt % 2 == 0:
                                nc.vector.tensor_copy(lsb[:pm, mt, :nw], ps_l[:pm, :nw])
                            else:
                                nc.scalar.copy(lsb[:pm, mt, :nw], ps_l[:pm, :nw])
                        nc.sync.dma_start(
                            out=d_out[:, nv * 512:nv * 512 + nw].rearrange(
                                "(m p) v -> p m v", p=128),
                            in_=lsb[:, :, :nw])

    nc.finalize()
    return nc


_PROG_CACHE = {}


def _get_program(reps: int = 1):
    if reps not in _PROG_CACHE:
        _PROG_CACHE[reps] = build_program(reps)
    return _PROG_CACHE[reps]


def _bf(a):
    return np.ascontiguousarray(a).astype(ml_dtypes.bfloat16)


def _prep_shared(inputs):
    emb = np.ascontiguousarray(inputs["emb"], dtype=np.float32)
    Wih = np.asarray(inputs["Wih"], dtype=np.float32)[PERM]
    Whh = np.asarray(inputs["Whh"], dtype=np.float32)[PERM]
    bih = np.asarray(inputs["bih"], dtype=np.float32)[PERM]
    bhh = np.asarray(inputs["bhh"], dtype=np.float32)[PERM]
    W1 = np.asarray(inputs["W1"], dtype=np.float32)
    W2 = np.asarray(inputs["W2"], dtype=np.float32)
    Wout = np.asarray(inputs["Wout"], dtype=np.float32)

    WhhT = Whh.T  # [G, 3G] (gate dim permuted)
    wih_p = np.zeros((EP, G3), np.float32)
    wih_p[:E] = Wih.T
    wout_p = np.zeros((EP, V), np.float32)
    wout_p[:E] = Wout.T
    b2_p = np.zeros(EP, np.float32)
    b2_p[:E] = np.asarray(inputs["b2"], dtype=np.float32)

    return {
        "emb": emb,
        "whh_h": _bf(WhhT[:H]),
        "whh_s": _bf(WhhT[H:]),
        "wih": _bf(wih_p),
        "w1": _bf(W1.T),
        "w2": _bf(W2.T),
        "wout": _bf(wout_p),
        "b1t": np.ascontiguousarray(
            np.asarray(inputs["b1"], np.float32).reshape(H // 128, 128).T),
        "biht": np.ascontiguousarray(bih.reshape(M3, 128).T),
        "bhht": np.ascontiguousarray(bhh.reshape(M3, 128).T),
        "b2t": np.ascontiguousarray(b2_p.reshape(EP // 128, 128).T),
    }


def make_in_maps(inputs):
    shared = _prep_shared(inputs)
    x = np.asarray(inputs["x"]).astype(np.int32)          # [B, T]
    ses = np.asarray(inputs["ses_encoding"], np.float32)[0]  # [B, SH]
    in_maps = []
    for c in range(NCORES):
        bs = slice(c * BL, (c + 1) * BL)
        xf = np.zeros(NTP, np.int32)
        xf[:NT] = x[bs].T.reshape(-1)  # t-major
        m = dict(shared)
        m["xw"] = np.ascontiguousarray(xf.reshape(7, 128).T)
        m["sesenc"] = _bf(ses[bs].T)
        in_maps.append(m)
    return in_maps


def run(inputs, reps: int = 1, **kwargs):
    nc = _get_program(reps)
    in_maps = make_in_maps(inputs)
    res = run_bass_kernel_spmd(nc, in_maps, core_ids=list(range(NCORES)), **kwargs)
    out = np.concatenate(
        [res.results[c]["out"][:NT].astype(np.float32).reshape(T, BL, V)
         .transpose(1, 0, 2) for c in range(NCORES)], axis=0)
    return np.ascontiguousarray(out)


def kernel(**inputs) -> np.ndarray:
    return run(inputs)
